# revision 1
# baseline (speedup 1.0000x reference)
"""Trainium2 Bass kernel for the 25-step spiking MLP (784 -> 1000 -> 10).

Data-parallel over batch: 4096 rows split across 8 NeuronCores (512 each).

Layer-1 state is kept as E = 2*(mem1 - 1), so the LIF step becomes
    E' = beta*E - sgn + cur1E      sgn = Sign(E') in {-1,+1}  (spk=(sgn+1)/2)
with cur1E = 2*(x@W1.T + b1) - 2*(1-beta) - 1 folded into the fc1 weights.
Per step: one DVE scalar_tensor_tensor (beta-decay minus spike), one
tensor_tensor add of the constant cur1E (split DVE/GPSIMD), one ScalarE Sign
producing fp16 +/-1 spikes. The hidden dim is split into 8 independent
column-group chains so the recurrence pipelines across engines, and the sgn
tensor rotates through a buffer pool so early groups run several steps ahead
(hiding the fp32 fc1 prologue).

fc2 consumes the +/-1 fp16 spikes with W2 split into fp16 hi+lo parts
(~fp32 accuracy at 1 cycle/row): rhs weights are 0.5*W2.T and the
always-firing pad unit's weight row carries b2 + 0.5*sum_h W2.T[h].
cur2 lands as [10, 512]; ScalarE copies it to SBUF and TensorE transposes it
to the [128b x (4bt*10o)] layout where layer-2 LIF runs exactly as the
reference; spk2/mem2 records DMA out each step.
"""

import numpy as np

import concourse.bass as bass
import concourse.mybir as mybir
import concourse.tile as tile
from concourse import bacc
from concourse.bass_utils import run_bass_kernel_spmd

F32 = mybir.dt.float32
F16 = mybir.dt.float16
ALU = mybir.AluOpType
ACTF = mybir.ActivationFunctionType

N_CORES = 8
B = 4096
PB = B // N_CORES          # 512 batch rows per core
INP = 784
KA = INP + 1               # ones-row folds the bias in
HID = 1000
HIDP = 1024                # padded hidden (8 x 128)
OUT = 10
T = 25
BETA = 0.95
BIG = 60000.0    # fp16-representable

NHT = HIDP // 128          # 8 hidden-tile column groups (512 cols each)
NBT = PB // 128            # 4 batch partition-tiles
KSPLITS = [(i * 128, min(128, KA - i * 128)) for i in range((KA + 127) // 128)]

POOL_GROUPS = 6
SPLIT_G = (5, 6)       # these groups' adds are half Pool / half DVE
SIGN_PAIR = 2
STT_PAIR = 1
SGN_BUFS = 8               # sgn rotation depth (layer-1 run-ahead)


def _build_program():
    nc = bacc.Bacc("TRN2", target_bir_lowering=False, debug=False,
                   enable_partition_id=False)

    xt_d = nc.dram_tensor("xt", [KA, PB], F32, kind="ExternalInput")
    w1t_d = nc.dram_tensor("w1t", [KA, HIDP], F32, kind="ExternalInput")
    w2h_d = nc.dram_tensor("w2h", [HIDP, OUT], F16, kind="ExternalInput")
    w2l_d = nc.dram_tensor("w2l", [HIDP, OUT], F16, kind="ExternalInput")
    idt_d = nc.dram_tensor("idt", [OUT, OUT], F32, kind="ExternalInput")
    ospk_d = nc.dram_tensor("ospk", [T, PB, OUT], F32, kind="ExternalOutput")
    omem_d = nc.dram_tensor("omem", [T, PB, OUT], F32, kind="ExternalOutput")

    with tile.TileContext(nc) as tc:
        with (
            tc.tile_pool(name="state", bufs=1) as state,
            tc.tile_pool(name="sgnp", bufs=SGN_BUFS) as sgnp,
            tc.tile_pool(name="l2", bufs=3) as l2p,
            tc.tile_pool(name="psum", bufs=2, space="PSUM") as psp,
            tc.tile_pool(name="psum3", bufs=3, space="PSUM") as psp3,
        ):
            # ---- load inputs ----
            xt_t, w1t_t = [], []
            for i, (k0, kk) in enumerate(KSPLITS):
                xk = state.tile([kk, PB], F32, tag=f"xt{i}")
                nc.sync.dma_start(xk[:], xt_d.ap()[k0:k0 + kk, :])
                xt_t.append(xk)
                wk = state.tile([kk, HIDP], F32, tag=f"w1t{i}")
                nc.sync.dma_start(wk[:], w1t_d.ap()[k0:k0 + kk, :])
                w1t_t.append(wk)
            w2h_t = []
            w2l_t = []
            for j in range(NHT):
                wj = state.tile([128, OUT], F16, tag=f"w2h{j}")
                nc.sync.dma_start(wj[:], w2h_d.ap()[j * 128:(j + 1) * 128, :])
                w2h_t.append(wj)
                wl = state.tile([128, OUT], F16, tag=f"w2l{j}")
                nc.sync.dma_start(wl[:], w2l_d.ap()[j * 128:(j + 1) * 128, :])
                w2l_t.append(wl)

            # ---- persistent state [128, NHT*PB]; group g = cols [g*PB,(g+1)*PB) ----
            cur1 = state.tile([128, NHT * PB], F32, tag="cur1")
            est = state.tile([128, NHT * PB], F32, tag="est")

            # 10x10 identity for PE record transposes
            idt = state.tile([OUT, OUT], F32, tag="idt")
            nc.sync.dma_start(idt[:], idt_d.ap())

            # ---- fc1: cur1E (weights pre-scaled on host), exact fp32 ----
            for j in range(NHT):
                ps = psp.tile([128, PB], F32, tag="fc1")
                for i, (k0, kk) in enumerate(KSPLITS):
                    nc.tensor.matmul(
                        ps[:],
                        w1t_t[i][:, j * 128:(j + 1) * 128],
                        xt_t[i][:],
                        start=(i == 0),
                        stop=(i == len(KSPLITS) - 1),
                    )
                half = PB // 2
                nc.scalar.copy(cur1[:, j * PB:j * PB + half], ps[:, :half])
                nc.vector.tensor_copy(cur1[:, j * PB + half:(j + 1) * PB],
                                      ps[:, half:])

            for g0 in range(0, NHT, STT_PAIR):
                lo, hi = g0 * PB, (g0 + STT_PAIR) * PB
                nc.vector.memset(est[:, lo:hi], -2.0)  # E_0 = 2*(mem-1)|mem=0

            sgn_prev = sgnp.tile([128, NHT * PB], F16, tag="sgn")
            for g0 in range(0, NHT, STT_PAIR):
                lo, hi = g0 * PB, (g0 + STT_PAIR) * PB
                nc.vector.memset(sgn_prev[:, lo:hi], -1.0)  # spk_0 = 0

            mem2_prev = l2p.tile([128, NBT * OUT], F32, tag="mem2")
            spk2_prev = l2p.tile([128, NBT * OUT], F32, tag="spk2")
            nc.vector.memset(mem2_prev[:], 0.0)
            nc.vector.memset(spk2_prev[:], 0.0)

            def gs(buf, g):
                return buf[:, g * PB:(g + 1) * PB]

            def l2_block(c2t, t):
                """Layer-2 LIF + records for step t (software-pipelined)."""
                nonlocal mem2_prev, spk2_prev
                c2s = l2p.tile([OUT, PB], F32, tag="c2s")
                nc.scalar.copy(c2s[:], c2t[:])
                c2 = psp.tile([128, NBT * OUT], F32, tag="c2")
                for bt in range(NBT):
                    nc.tensor.transpose(c2[:, bt * OUT:(bt + 1) * OUT],
                                        c2s[:, bt * 128:(bt + 1) * 128],
                                        idt[:])
                mem2 = l2p.tile([128, NBT * OUT], F32, tag="mem2")
                spk2 = l2p.tile([128, NBT * OUT], F32, tag="spk2")
                nc.vector.scalar_tensor_tensor(mem2[:], mem2_prev[:], BETA,
                                               spk2_prev[:], ALU.mult,
                                               ALU.subtract)
                nc.vector.tensor_tensor(mem2[:], mem2[:], c2[:], ALU.add)
                nc.vector.tensor_scalar(spk2[:], mem2[:], 1.0, None, ALU.is_gt)
                base = (t - 1) * PB * OUT
                dims = [[OUT, 128], [128 * OUT, NBT], [1, OUT]]
                nc.sync.dma_start(bass.AP(ospk_d, base, [d[:] for d in dims]),
                                  spk2[:])
                nc.sync.dma_start(bass.AP(omem_d, base, [d[:] for d in dims]),
                                  mem2[:])
                mem2_prev, spk2_prev = mem2, spk2

            pend = []  # (c2t psum tile, step) awaiting layer-2 processing

            # ---- time loop (fully unrolled; groups pipeline across engines) ----
            for t in range(1, T + 1):
                # chain-major emission: each pair runs STT -> adds -> Sign so
                # the four pair-chains stagger across DVE/Pool/ACT instead of
                # convoying phase-by-phase
                sgn = sgnp.tile([128, NHT * PB], F16, tag="sgn")
                for lo_g in range(NHT):
                    hi_g = lo_g + 1
                    lo, hi = lo_g * PB, hi_g * PB
                    # E = beta*E - sgn_{t-1}
                    nc.vector.scalar_tensor_tensor(est[:, lo:hi], est[:, lo:hi],
                                                   BETA, sgn_prev[:, lo:hi],
                                                   ALU.mult, ALU.subtract)
                    for g in range(lo_g, hi_g):
                        # E += cur1E (group SPLIT_G split between Pool and DVE)
                        if g in SPLIT_G:
                            h2 = PB // 2
                            a, b = g * PB, g * PB + h2
                            c = (g + 1) * PB
                            nc.gpsimd.tensor_tensor(est[:, a:b], est[:, a:b],
                                                    cur1[:, a:b], ALU.add)
                            nc.vector.tensor_tensor(est[:, b:c], est[:, b:c],
                                                    cur1[:, b:c], ALU.add)
                        else:
                            eng = nc.gpsimd if g < POOL_GROUPS else nc.vector
                            eng.tensor_tensor(gs(est, g), gs(est, g),
                                              gs(cur1, g), ALU.add)
                for g0 in range(0, 4, SIGN_PAIR):
                    lo, hi = g0 * PB, (g0 + SIGN_PAIR) * PB
                    nc.scalar.activation(sgn[:, lo:hi], est[:, lo:hi],
                                         ACTF.Sign)
                for g0 in (4, 5, 6, 7):
                    lo, hi = g0 * PB, (g0 + 1) * PB
                    nc.scalar.activation(sgn[:, lo:hi], est[:, lo:hi],
                                         ACTF.Sign)

                # layer-2 lagging two steps: transposes/copies have long-ready
                # inputs, so no engine stalls behind fc2_t
                if len(pend) >= 2:
                    l2_block(*pend.pop(0))
                # fc2: cur2T[o, b] = sum_j (w2h_j + w2l_j).T @ sgn_j  (fp16)
                c2t = psp3.tile([OUT, PB], F32, tag="c2t")
                for j in range(NHT):
                    nc.tensor.matmul(c2t[:], w2h_t[j][:], gs(sgn, j),
                                     start=(j == 0), stop=False)
                for j in range(NHT):
                    nc.tensor.matmul(c2t[:], w2l_t[j][:], gs(sgn, j),
                                     start=False, stop=(j == NHT - 1))
                pend.append((c2t, t))
                sgn_prev = sgn
            for p_ in pend:
                l2_block(*p_)

    nc.compile()
    return nc


_NC_CACHE = None


def _prep(x, W1, b1, W2, b2):
    """Host-side input prep shared by all cores."""
    # fc1 produces cur1E = 2*(x@W1.T + b1) - 2*(1-BETA) - 1 directly
    w1t = np.zeros((KA, HIDP), np.float32)
    w1t[:INP, :HID] = 2.0 * W1.T
    w1t[INP, :HID] = 2.0 * b1 - 2.0 * (1.0 - BETA) - 1.0
    w1t[INP, HID] = BIG          # pad unit 1000: sgn=+1 always
    w1t[INP, HID + 1:] = -BIG    # other pad units: sgn=-1 always
    # fc2 on +/-1 spikes: 0.5*W2.T, always-row carries b2 + 0.5*sum(W2.T);
    # fp16 hi + lo split for ~fp32 matmul accuracy at full PE rate
    w2t = np.zeros((HIDP, OUT), np.float32)
    w2t[:HID] = 0.5 * W2.T
    w2t[HID] = b2 + 0.5 * W2.T.sum(axis=0)
    w2h = w2t.astype(np.float16)
    w2l = (w2t - w2h.astype(np.float32)).astype(np.float16)
    xt = np.concatenate([x.T, np.ones((1, x.shape[0]), np.float32)], axis=0)
    return w1t, w2h, w2l, xt


def kernel(x, W1, b1, W2, b2):
    global _NC_CACHE
    x = np.ascontiguousarray(np.asarray(x, np.float32))
    W1 = np.asarray(W1, np.float32)
    b1 = np.asarray(b1, np.float32)
    W2 = np.asarray(W2, np.float32)
    b2 = np.asarray(b2, np.float32)

    w1t, w2h, w2l, xt = _prep(x, W1, b1, W2, b2)

    if _NC_CACHE is None:
        _NC_CACHE = _build_program()
    nc = _NC_CACHE

    in_maps = []
    for c in range(N_CORES):
        sl = slice(c * PB, (c + 1) * PB)
        in_maps.append({
            "xt": np.ascontiguousarray(xt[:, sl]),
            "w1t": w1t,
            "w2h": w2h,
            "w2l": w2l,
            "idt": np.eye(OUT, dtype=np.float32),
        })

    res = run_bass_kernel_spmd(nc, in_maps, core_ids=list(range(N_CORES)))
    kernel.last_results = res

    ospk = np.empty((T, B, OUT), np.float32)
    omem = np.empty((T, B, OUT), np.float32)
    for c in range(N_CORES):
        sl = slice(c * PB, (c + 1) * PB)
        ospk[:, sl, :] = res.results[c]["ospk"]
        omem[:, sl, :] = res.results[c]["omem"]
    return ospk, omem



# revision 29
# speedup vs baseline: 1.4775x; 1.4775x over previous
"""Trainium2 Bass kernel for the 25-step spiking MLP (784 -> 1000 -> 10).

Data-parallel over batch: 4096 rows split across 8 NeuronCores (512 each).

Layer-1 state is the negated membrane H = -mem1 (init 0).  A custom fused
DVE op
    LIF_NEG_ANT: out = (Src0*C0 - Src1) + (Src0 < C1)
runs a whole LIF step (decay + drive + subtract-reset) in ONE Vector-engine
pass:  H' = beta*H - cur1 + [H < -1]   (spike s = [mem > 1] = [H < -1]).
The same form fits two GPSIMD scalar_tensor_tensor ops
    X = beta*H - cur1;  H' = [H < -1] + X,
so the 4096 hidden x batch state columns split across both engines:
g0 + g3..g7 (3072 cols, three chained column-block chains) on DVE and
g1,g2 (1024 cols) on GPSIMD.  Chains start staggered as their fc1 slice
lands (wavefront skews) so nothing waits for the full fc1.

Spikes are never materialized for fc2.  Since s_t = beta*mem_t + cur1
- mem_{t+1} and P_t := W2 @ mem_t = (-W2) @ H_t,
    cur2_t = beta*P_t + Q - P_{t+1},   Q = W2 @ cur1 + b2,
so fc2 is 36 tiny fp32 matmuls per step (output free size 10) plus two
small scalar_tensor_tensor ops on [128, 40].  Layer-2 LIF uses the
positive-form twin op LIF_THR_ANT (out = (Src0*C0 + Src1) - (Src0 > C1));
records are spk2 = (mem2 > 1) and mem2 itself, written side by side and
DMA'd out once per step.

fc1 runs in fp16 hi/lo (3 matmul passes, ~2^-22 accurate, single-group
units), inputs arrive as a handful of big packed DMAs.
"""

import os

import numpy as np

import concourse.bass as bass
import concourse.mybir as mybir
import concourse.tile as tile
import concourse.dve_ops as dve_ops
from concourse import bacc
from concourse.bass_utils import run_bass_kernel_spmd
from concourse.dve_spec import (
    Spec, Src0, Src1, C0, C1, lower as dve_lower,
)
from concourse.dve_uop import DveOpSpec
from concourse.dve_table_gen import dve_ver_for, free_opcode_rows

F32 = mybir.dt.float32
F16 = mybir.dt.float16
ALU = mybir.AluOpType

N_CORES = 8
B = 4096
PB = B // N_CORES          # 512 batch rows per core
INP = 784
KA = INP + 1               # ones-row folds the bias in
HID = 1000
HIDP = 1024                # padded hidden (8 x 128)
OUT = 10
T = 25
TS = T + 1                 # one extra state step recovers s_25
BETA = 0.95

NHT = HIDP // 128          # 8 hidden-tile groups (512 state cols each)
NBT = PB // 128            # 4 batch partition-tiles
NK = (KA + 127) // 128     # 7 K-splits
KSPLITS = [(i * 128, min(128, KA - i * 128)) for i in range(NK)]

P_SLOTS_PER_BANK = 12
P_BANKS = 3

# layer-1 chains: (name, engine, col0, width, enter wave, rate, final skew,
#                  state bufs).  "dve" = H-form custom op; "epool" = E-form
#   split ACT(decay, Sign) + Pool(two tensor_tensor ops).
E_LO, E_HI = 512, 1280     # E-form state columns (g1 + g2/bt0-1)
CHAINS = (
    ("c0", "dve", 0, 512, 1, 2, 0, 10),
    ("ce1", "epool", 512, 384, 3, 1, 1, 4),
    ("ce2", "epool", 896, 384, 5, 1, 2, 4),
    ("c1", "dve", 1280, 768, 10, 2, 2, 8),
    ("c2", "dve", 2048, 1024, 13, 2, 3, 8),
    ("c3", "dve", 3072, 1024, 16, 2, 4, 8),
)
# fc1 group -> wave before which its unit is emitted (0 = before the loop)
FC1_WAVE = {0: 0, 1: 0, 2: 2, 3: 6, 4: 8, 5: 11, 6: 13, 7: 15}


def _build_sched():
    """Per chain: {wave: [t, ...]}; plus per-t first/last chain and l2 waves."""
    sched = {}
    wave_of = {}           # (name, t) -> wave
    for name, eng, c0_, wd, enter, rate, fsk, nb in CHAINS:
        t, w, m = 1, enter, {}
        while t <= TS:
            n = 1 if t > w - fsk - 1 else rate
            n = min(n, TS - t + 1)
            if t <= w - fsk:
                n = max(n, 1)
            steps = list(range(t, t + n))
            m[w] = steps
            for s in steps:
                wave_of[(name, s)] = w
            t += n
            w += 1
        sched[name] = m
    names = [c[0] for c in CHAINS]
    first, last, wdone = {}, {}, {}
    for t in range(1, TS + 1):
        key = sorted(names, key=lambda n: (wave_of[(n, t)], names.index(n)))
        first[t], last[t] = key[0], key[-1]
        wdone[t] = max(wave_of[(n, t)] for n in names)
    return sched, first, last, wdone


SCHED, P_FIRST, P_LAST, W_DONE = _build_sched()
assert all(P_FIRST[t] == "c0" and P_LAST[t] == "c3"
           for t in range(1, TS + 1)), (P_FIRST, P_LAST)
N_WAVES = max(max(m) for m in SCHED.values())
PC_W = {t: W_DONE[t] + 1 for t in range(1, TS + 1)}
L2_HEAD_W = {t: W_DONE[t + 1] + 2 for t in range(1, T + 1)}
L2_TAIL_W = {t: W_DONE[t + 1] + 3 for t in range(1, T + 1)}
POOL_LAST_W = max(SCHED["ce2"])  # after this, l2 head ops run on DVE
N_WAVES = max(N_WAVES, max(L2_TAIL_W.values()))



def _register_op(name, spec_body, ref):
    for op in dve_ops.OPS:
        if op.name == name:
            return op
    spec = Spec(body=spec_body, reference=ref)
    ver = dve_ver_for("TRN2")
    used = set(dve_ops._SUB_OPCODE_FOR_NAME.values())
    row = next(r for r in free_opcode_rows("TRN2") if r not in used)
    sha = DveOpSpec(name=name, opcode=row, uops=dve_lower(spec, ver=ver),
                    rd1_en=True).sha(ver)
    op = dve_ops.DveOp(name, spec, subdim=False, uops_sha={ver: sha})
    dve_ops.OPS.append(op)
    dve_ops._SUB_OPCODE_FOR_NAME[name] = row
    dve_ops.CUSTOM_DVE_SPECS[name] = spec
    return op


def _ref_thr(in0, in1, c0, c1, c2):
    a = in0.astype(np.float32)
    return (a * np.float32(c0) + in1.astype(np.float32)) - (a > c1).astype(
        np.float32)


def _ref_neg(in0, in1, c0, c1, c2):
    a = in0.astype(np.float32)
    return (a * np.float32(c0) - in1.astype(np.float32)) + (a < c1).astype(
        np.float32)


LIF_THR = _register_op("LIF_THR_ANT", (Src0 * C0 + Src1) - (Src0 > C1),
                       _ref_thr)
LIF_NEG = _register_op("LIF_NEG_ANT", (Src0 * C0 - Src1) + (Src0 < C1),
                       _ref_neg)

# packed input widths
XW = 1024                  # per k: [512 xh | 512 xl]
GW = 256                   # per (g, k): [128 w1h | 128 w1l]


def _build_program():
    nc = bacc.Bacc("TRN2", target_bir_lowering=False, debug=False,
                   enable_partition_id=False)

    x_d = nc.dram_tensor("xhl", [128, NK * XW], F16, kind="ExternalInput")
    w_d = nc.dram_tensor("w1hl", [128, NHT * NK * GW], F16,
                         kind="ExternalInput")
    w2_d = nc.dram_tensor("w2all", [128, 3 * NHT * OUT], F32,
                          kind="ExternalInput")
    on_d = nc.dram_tensor("on", [1, 128 + NBT * OUT], F32,
                          kind="ExternalInput")
    orec_d = nc.dram_tensor("orec", [T, 2, PB, OUT], F32,
                            kind="ExternalOutput")
    dbg = os.environ.get("KDBG") == "1"
    if dbg:
        dct_d = nc.dram_tensor("dct", [128, NHT * PB], F32,
                               kind="ExternalOutput")
        dq_d = nc.dram_tensor("dq", [128, NBT * OUT], F32,
                              kind="ExternalOutput")
        dp_d = nc.dram_tensor("dp", [3, 128, NBT * OUT], F32,
                              kind="ExternalOutput")
        ds_d = nc.dram_tensor("ds", [128, NHT * PB], F32,
                              kind="ExternalOutput")

    with tile.TileContext(nc) as tc:
        pools = {}
        with (
            tc.tile_pool(name="state", bufs=1) as state,
            tc.tile_pool(name="gc0", bufs=10) as gc0,
            tc.tile_pool(name="ge1", bufs=4) as ge1,
            tc.tile_pool(name="ge2", bufs=4) as ge2,
            tc.tile_pool(name="gc1", bufs=8) as gc1,
            tc.tile_pool(name="gc2", bufs=8) as gc2,
            tc.tile_pool(name="gc3", bufs=8) as gc3,
            tc.tile_pool(name="sgn", bufs=3) as sgnp,
            tc.tile_pool(name="phx", bufs=2) as phxp,
            tc.tile_pool(name="l2", bufs=4) as l2p,
            tc.tile_pool(name="rec", bufs=10) as recp,
            tc.tile_pool(name="pcr", bufs=6) as pcrp,
            tc.tile_pool(name="psf", bufs=2, space="PSUM") as psf,
            tc.tile_pool(name="psp", bufs=1, space="PSUM") as psp,
        ):
            pools.update(c0=gc0, ce1=ge1, ce2=ge2, c1=gc1, c2=gc2,
                         c3=gc3)
            # ---- loads: first the slices fc1(g0), fc1(g1) need ----
            w1 = state.tile([128, NHT * NK * GW], F16, tag="w1hl")
            nc.sync.dma_start(w1[:, :NK * GW], w_d.ap()[:, :NK * GW])
            xhl = state.tile([128, NK * XW], F16, tag="xhl")
            nc.sync.dma_start(xhl[:, :4 * XW], x_d.ap()[:, :4 * XW])
            nc.sync.dma_start(xhl[:, 4 * XW:], x_d.ap()[:, 4 * XW:])
            for g in range(1, NHT):
                nc.sync.dma_start(
                    w1[:, g * NK * GW:(g + 1) * NK * GW],
                    w_d.ap()[:, g * NK * GW:(g + 1) * NK * GW])
            w2all = state.tile([128, 3 * NHT * OUT], F32, tag="w2all")
            nc.sync.dma_start(w2all[:], w2_d.ap())
            w2p_t = [w2all[:, j * OUT:(j + 1) * OUT] for j in range(NHT)]
            w2n_t = [w2all[:, (NHT + j) * OUT:(NHT + j + 1) * OUT]
                     for j in range(NHT)]
            w2h_t = [w2all[:, (2 * NHT + j) * OUT:(2 * NHT + j + 1) * OUT]
                     for j in range(NHT)]
            msc = state.tile([1, 128 + NBT * OUT], F32, tag="msc")
            nc.sync.dma_start(msc[:], on_d.ap())
            ones = msc[:, :128]

            ct = state.tile([128, NHT * PB], F32, tag="ct")

            # ---- zero-init chain states on idle ACT ----
            cur = {}
            sgn_cur = {}
            for name, eng, c0_, w_, ent, rate, fsk, nb in CHAINS:
                t0 = pools[name].tile([128, w_], F32, tag=name)
                if eng == "epool":
                    nc.vector.memset(t0[:], -2.0)      # E_0 = 2*(mem-1)
                    s0_ = sgnp.tile([128, w_], F32, tag="s" + name)
                    nc.vector.memset(s0_[:], -1.0)     # Sign(E_0)
                    sgn_cur[name] = s0_
                else:
                    nc.scalar.memzero(t0[:])
                cur[name] = t0
            g2m = recp.tile([128, 2 * NBT * OUT], F32, tag="g2")
            nc.scalar.memzero(g2m[:])
            cur["g2"] = g2m[:, NBT * OUT:]

            # ---- fc1 (fp16 hi/lo, 3 passes), one group per unit ----
            def fc1(g):
                ps = psf.tile([128, PB], F32, tag="fc1")
                n = 3 * NK
                i = 0
                for wlo, xlo in ((False, False), (False, True), (True, False)):
                    for k in range(NK):
                        kk = KSPLITS[k][1]
                        wc = (g * NK + k) * GW + (128 if wlo else 0)
                        xc = k * XW + (512 if xlo else 0)
                        nc.tensor.matmul(
                            ps[:], w1[0:kk, wc:wc + 128],
                            xhl[0:kk, xc:xc + 512],
                            start=(i == 0), stop=(i == n - 1),
                        )
                        i += 1
                c0_ = g * PB
                c1_ = (g + 1) * PB
                lo = max(c0_, E_LO)
                hi = min(c1_, E_HI)
                if lo >= hi:                      # plain H-form group
                    nc.scalar.copy(ct[:, c0_:c1_], ps[:])
                else:
                    if c0_ < lo:
                        nc.scalar.copy(ct[:, c0_:lo], ps[:, :lo - c0_])
                    nc.scalar.activation(
                        ct[:, lo:hi], ps[:, lo - c0_:hi - c0_],
                        mybir.ActivationFunctionType.Identity,
                        bias=ebias[:], scale=2.0)
                    if hi < c1_:
                        nc.scalar.copy(ct[:, hi:c1_], ps[:, hi - c0_:])

            # per-partition constant 2*beta-3 for the E-column fc1 copies
            ebias = state.tile([128, 1], F32, tag="ebias")
            nc.vector.memset(ebias[:], 2.0 * BETA - 3.0)

            for g, fw in FC1_WAVE.items():
                if fw == 0:
                    fc1(g)

            # ---- PSUM P-slot ring ----
            # PSUM P banks: a start=True matmul zeroes a whole bank on HW,
            # so pre-zero each bank once and use accumulate-only matmuls.
            pbanks = []
            for i in range(P_BANKS):
                pb_i = psp.tile([128, P_SLOTS_PER_BANK * NBT * OUT], F32,
                                tag=f"pb{i}")
                nc.vector.memset(pb_i[:], 0.0)
                pbanks.append(pb_i)
            n_slots = P_BANKS * P_SLOTS_PER_BANK

            def pslot(idx):
                c0_ = (idx % P_SLOTS_PER_BANK) * NBT * OUT
                return pbanks[idx // P_SLOTS_PER_BANK], c0_

            # ---- Q = W2 @ cur1 + b2 (emitted after the last fc1 unit) ----
            qsb = state.tile([128, NBT * OUT], F32, tag="qsb")

            def emit_q():
                qtl, qc0 = pslot(n_slots - 1)
                for g in range(NHT):
                    for bt in range(NBT):
                        col = g * PB + bt * 128
                        w2q = w2h_t[g] if E_LO <= col < E_HI else w2p_t[g]
                        nc.tensor.matmul(
                            qtl[:, qc0 + bt * OUT:qc0 + (bt + 1) * OUT],
                            ct[:, col:col + 128], w2q,
                            start=False, stop=False, skip_group_check=True,
                        )
                for bt in range(NBT):
                    nc.tensor.matmul(
                        qtl[:, qc0 + bt * OUT:qc0 + (bt + 1) * OUT],
                        ones, msc[:, 128 + bt * OUT:128 + (bt + 1) * OUT],
                        start=False, stop=True, skip_group_check=True,
                    )
                nc.scalar.copy(qsb[:], qtl[:, qc0:qc0 + NBT * OUT])
                if dbg:
                    nc.sync.dma_start(dq_d.ap(), qsb[:])

            # ---- P contributions: H_t slices, negated W2 ----
            def p_parts(t, name, st, base, width):
                ptl, pc0 = pslot(t - 1)
                for off in range(0, width, 128):
                    col = base + off
                    g, bt = col // PB, (col % PB) // 128
                    w2 = w2h_t[g] if E_LO <= col < E_HI else w2n_t[g]
                    nc.tensor.matmul(
                        ptl[:, pc0 + bt * OUT:pc0 + (bt + 1) * OUT],
                        st[:, off:off + 128], w2,
                        start=False,
                        stop=(name == P_LAST[t] and off >= width - 512),
                        skip_group_check=True,
                    )

            def dve_step(name, base, width, t):
                nxt = pools[name].tile([128, width], F32, tag=name)
                nc.vector._custom_dve(LIF_NEG, out=nxt[:], in0=cur[name][:],
                                      in1=ct[:, base:base + width],
                                      s0=BETA, s1=-1.0)
                cur[name] = nxt
                if dbg and t == 1:
                    nc.sync.dma_start(ds_d.ap()[:, base:base + width], nxt[:])
                p_parts(t, name, nxt, base, width)

            def e_step(name, base, width, t):
                """E' = beta*E + ctE - Sign(E): ACT decay+Sign, Pool adds."""
                c = cur[name]
                u = phxp.tile([128, width], F32, tag="u" + name)
                nc.scalar.activation(u[:], c[:],
                                     mybir.ActivationFunctionType.Copy,
                                     scale=BETA)
                v = phxp.tile([128, width], F32, tag="v" + name)
                nc.gpsimd.tensor_tensor(v[:], u[:], ct[:, base:base + width],
                                        ALU.add)
                nxt = pools[name].tile([128, width], F32, tag=name)
                nc.gpsimd.tensor_tensor(nxt[:], v[:], sgn_cur[name][:],
                                        ALU.subtract)
                sg = sgnp.tile([128, width], F32, tag="s" + name)
                nc.scalar.activation(sg[:], nxt[:],
                                     mybir.ActivationFunctionType.Sign)
                cur[name] = nxt
                sgn_cur[name] = sg
                if dbg and t == 1:
                    nc.sync.dma_start(ds_d.ap()[:, base:base + width], nxt[:])
                p_parts(t, name, nxt, base, width)

            pc_sb = {}

            def pc_copy(t):
                """ACT moves completed P_t PSUM -> SBUF (GPSIMD can't read
                PSUM on real hardware)."""
                ptl, pc0 = pslot(t - 1)
                pc = pcrp.tile([128, NBT * OUT], F32, tag="pc")
                nc.scalar.copy(pc[:], ptl[:, pc0:pc0 + NBT * OUT])
                pc_sb[t] = pc
                if dbg and t <= 3:
                    nc.sync.dma_start(dp_d.ap()[t - 1], pc[:])

            def l2_head(t, on_pool):
                """cur2_t = beta*P_t + Q - P_{t+1} in TT/scaled-copy form."""
                d = l2p.tile([128, NBT * OUT], F32, tag="d")
                u2 = l2p.tile([128, NBT * OUT], F32, tag="u2")
                nc.scalar.activation(u2[:], pc_sb.pop(t)[:],
                                     mybir.ActivationFunctionType.Copy,
                                     scale=BETA)
                c2 = l2p.tile([128, NBT * OUT], F32, tag="c2")
                if on_pool:
                    nc.gpsimd.tensor_tensor(d[:], qsb[:], pc_sb[t + 1][:],
                                            ALU.subtract)
                    nc.gpsimd.tensor_tensor(c2[:], u2[:], d[:], ALU.add)
                else:
                    nc.vector.tensor_tensor(d[:], qsb[:], pc_sb[t + 1][:],
                                            ALU.subtract)
                    nc.vector.tensor_tensor(c2[:], u2[:], d[:], ALU.add)
                return c2

            def l2_tail(t, c2, spk_of):
                rec = recp.tile([128, 2 * NBT * OUT], F32, tag="g2")
                g2n = rec[:, NBT * OUT:]
                nc.vector._custom_dve(LIF_THR, out=g2n, in0=cur["g2"],
                                      in1=c2[:], s0=BETA, s1=1.0)
                cur["g2"] = g2n
                if spk_of is not None:
                    prev_rec, pt, seng = spk_of
                    seng.tensor_scalar(prev_rec[:, :NBT * OUT],
                                       prev_rec[:, NBT * OUT:],
                                       1.0, None, ALU.is_gt)
                    base = (pt - 1) * 2 * PB * OUT
                    dims = [[OUT, 128], [PB * OUT, 2], [128 * OUT, NBT],
                            [1, OUT]]
                    nc.sync.dma_start(
                        bass.AP(orec_d, base, [d_[:] for d_ in dims]),
                        prev_rec[:])
                return rec

            # ---- wavefront loop ----
            pend_c2 = {}
            pend_rec = {}
            heads_at = {}
            tails_at = {}
            pcs_at = {}
            for t, w_ in L2_HEAD_W.items():
                heads_at.setdefault(w_, []).append(t)
            for t, w_ in L2_TAIL_W.items():
                tails_at.setdefault(w_, []).append(t)
            for t, w_ in PC_W.items():
                pcs_at.setdefault(w_, []).append(t)
            for w in range(1, N_WAVES + 1):
                for t in pcs_at.get(w, ()):
                    pc_copy(t)
                for t in heads_at.get(w, ()):
                    pend_c2[t] = l2_head(t, w <= POOL_LAST_W)
                for g, fw in FC1_WAVE.items():
                    if fw == w:
                        fc1(g)
                if w == max(FC1_WAVE.values()) + 1:
                    emit_q()
                    if dbg:
                        nc.sync.dma_start(dct_d.ap(), ct[:])
                for name, eng, c0_, wd, ent, rate, fsk, nb in CHAINS:
                    for t in SCHED[name].get(w, ()):
                        if eng == "dve":
                            dve_step(name, c0_, wd, t)
                        else:
                            e_step(name, c0_, wd, t)
                for tl in tails_at.get(w, ()):
                    seng = nc.gpsimd if w <= POOL_LAST_W else nc.vector
                    spk_of = (pend_rec.pop(tl - 1) + (seng,)) if tl - 1 in \
                        pend_rec else None
                    pend_rec[tl] = (l2_tail(tl, pend_c2.pop(tl), spk_of), tl)
            # flush the last record
            rec, lt = pend_rec.pop(T)
            nc.vector.tensor_scalar(rec[:, :NBT * OUT], rec[:, NBT * OUT:],
                                    1.0, None, ALU.is_gt)
            base = (lt - 1) * 2 * PB * OUT
            dims = [[OUT, 128], [PB * OUT, 2], [128 * OUT, NBT], [1, OUT]]
            nc.sync.dma_start(bass.AP(orec_d, base, [d_[:] for d_ in dims]),
                              rec[:])

    nc.compile()
    return nc


_NC_CACHE = None


def _prep(x, W1, b1, W2, b2):
    """Host-side packing shared by all cores (x shard applied per core)."""
    w1t = np.zeros((NK * 128, HIDP), np.float32)
    w1t[:INP, :HID] = W1.T
    w1t[INP, :HID] = b1
    w1h = w1t.astype(np.float16)
    w1l = (w1t - w1h.astype(np.float32)).astype(np.float16)
    w1hl = np.zeros((128, NHT * NK * GW), np.float16)
    for g in range(NHT):
        for k in range(NK):
            r = slice(k * 128, (k + 1) * 128)
            c = slice(g * 128, (g + 1) * 128)
            base = (g * NK + k) * GW
            w1hl[:, base:base + 128] = w1h[r, c]
            w1hl[:, base + 128:base + 256] = w1l[r, c]
    w2pad = np.zeros((HIDP, OUT), np.float32)
    w2pad[:HID] = W2.T
    w2all = np.zeros((128, 3 * NHT * OUT), np.float32)
    for g in range(NHT):
        w2all[:, g * OUT:(g + 1) * OUT] = w2pad[g * 128:(g + 1) * 128]
        w2all[:, (NHT + g) * OUT:(NHT + g + 1) * OUT] = \
            -w2pad[g * 128:(g + 1) * 128]
        w2all[:, (2 * NHT + g) * OUT:(2 * NHT + g + 1) * OUT] = \
            0.5 * w2pad[g * 128:(g + 1) * 128]
    # per-bt bias row: b2 + 0.5 * sum of W2 rows that are E-form for that bt
    qb = np.zeros((NBT, OUT), np.float32)
    for bt in range(NBT):
        ke = np.zeros(OUT, np.float32)
        for g in range(NHT):
            col = g * PB + bt * 128
            if E_LO <= col < E_HI:
                ke += w2pad[g * 128:(g + 1) * 128].sum(axis=0)
        qb[bt] = b2 + 0.5 * ke
    on = np.concatenate([np.ones(128, np.float32),
                         qb.ravel()])[None, :]
    xt = np.concatenate([x.T, np.ones((1, x.shape[0]), np.float32)], axis=0)
    xpad = np.zeros((NK * 128, B), np.float32)
    xpad[:KA] = xt
    return w1hl, w2all, on, xpad


def kernel(x, W1, b1, W2, b2):
    global _NC_CACHE
    x = np.ascontiguousarray(np.asarray(x, np.float32))
    W1 = np.asarray(W1, np.float32)
    b1 = np.asarray(b1, np.float32)
    W2 = np.asarray(W2, np.float32)
    b2 = np.asarray(b2, np.float32)

    w1hl, w2all, on, xpad = _prep(x, W1, b1, W2, b2)
    xh_all = xpad.astype(np.float16)
    xl_all = (xpad - xh_all.astype(np.float32)).astype(np.float16)

    if _NC_CACHE is None:
        _NC_CACHE = _build_program()
    nc = _NC_CACHE

    in_maps = []
    for c in range(N_CORES):
        sl = slice(c * PB, (c + 1) * PB)
        xhl = np.zeros((128, NK * XW), np.float16)
        for k in range(NK):
            r = slice(k * 128, (k + 1) * 128)
            xhl[:, k * XW:k * XW + 512] = xh_all[r, sl]
            xhl[:, k * XW + 512:(k + 1) * XW] = xl_all[r, sl]
        in_maps.append({
            "xhl": xhl,
            "w1hl": w1hl,
            "w2all": w2all,
            "on": on,
        })

    res = run_bass_kernel_spmd(nc, in_maps, core_ids=list(range(N_CORES)))
    kernel.last_results = res

    ospk = np.empty((T, B, OUT), np.float32)
    omem = np.empty((T, B, OUT), np.float32)
    for c in range(N_CORES):
        sl = slice(c * PB, (c + 1) * PB)
        rec = res.results[c]["orec"]
        ospk[:, sl, :] = rec[:, 0]
        omem[:, sl, :] = rec[:, 1]
    return ospk, omem
